# revision 16
# baseline (speedup 1.0000x reference)
"""DynamicW8A8Int8Linear on 8 Trainium2 NeuronCores (Bass/Tile).

Column-parallel (tensor-parallel on out_features): each core gets the full
activation x [8192, 4096] and a 1536-wide shard of weight / weight_scale /
bias; it computes its [8192, 1536] slice of the output. No communication.

Per-core pipeline (per 128-token m-tile):
  - DMA x tile [128, 4096] fp32 (HWDGE, ACT ring)
  - GpSimd: amax = max|x| over K;  DVE: xs = max(amax,1e-8)/127, inv = 1/xs
  - DVE: t = x*inv + MAGIC (fp32 magic-number round-half-even)
  - ACT: xq = t - MAGIC, cast to fp16 (ints in [-127,127] are exact)
  - DMA X-bar transpose (SP ring): xqT[p, c, m] = xq[m, c*128+p] -- the
    K-major layout the PE needs, at ~290 GB/s, zero PE/DVE cost
  - PE: 3 psum banks x 32 accumulating fp16 matmuls (512-wide moving op)
  - DVE: out = xs*acc + bias (one scalar_tensor_tensor per bank)
The weight shard is staged once: int8 -> fp16 cast during DMA (SWDGE),
multiplied by its per-channel scale (so the epilogue needs no ws), then
X-bar-transposed into the resident K-major wT tile reused by every m-tile.
fp16 keeps w*ws rounding at 2^-11 and the fp32-PSUM accumulation exact
enough for ~1e-3 relative error.
"""

import sys
from contextlib import ExitStack

import numpy as np

for p in ("/opt/trn_rl_repo", "/opt/pypackages"):
    if p not in sys.path:
        sys.path.append(p)

import orjson
import bass_rust
import concourse.bass as bass
import concourse.mybir as mybir
import concourse.tile as tile
from concourse.vector_clock import ScopedClock
from concourse.bass_utils import run_bass_kernel_spmd

# ---------------------------------------------------------------------------
# Workaround for the walrus build here, which accepts at most ONE sem-wait per
# instruction ("Too many sync wait commands" in setupSyncWait): split the Tile
# end-drain at emission time, and hoist excess waits from any instruction onto
# injected same-engine NoOps at serialization time (program order on the same
# engine makes that semantically identical).
# ---------------------------------------------------------------------------
MAX_WAITS = 1


def _drain_and_barrier_split(self, tick_clock, wait_clock):
    nc = self.nc
    drain_inst = nc.sync.drain()
    wait_clock.add_sem_waits(drain_inst.ins, ScopedClock({None: tick_clock.global_clock}))
    si = drain_inst.ins.sync_info
    waits = list(si.on_wait) if si is not None and si.on_wait else []
    if len(waits) > MAX_WAITS:
        si.on_wait = waits[:MAX_WAITS]
        drain_inst.ins.sync_info = si
        rest = waits[MAX_WAITS:]
        while rest:
            extra = nc.sync.drain()
            extra.ins.sync_info = bass_rust.SyncInfo(
                on_wait=rest[:MAX_WAITS], on_update=[])
            rest = rest[MAX_WAITS:]
    nc.all_engine_barrier()
    assert self.sems is not None
    popped = nc._tile_sem_poison_stack.pop()
    assert popped is self._sem_poison
    nc.clear_and_free_semaphores(list(self.sems.allocated().values()))
    nc.all_engine_barrier()


_split_counter = [0]


def _split_waits_json(raw: bytes) -> bytes:
    j = orjson.loads(raw)
    changed = [False]

    def fix_block(b):
        ins_list = b.get("instructions")
        if ins_list:
            new_list = []
            for ins in ins_list:
                si = ins.get("sync_info")
                waits = (si or {}).get("on_wait") or []
                if len(waits) > MAX_WAITS:
                    changed[0] = True
                    for w in waits[:-MAX_WAITS]:
                        _split_counter[0] += 1
                        new_list.append({
                            "name": f"WSPLIT-{_split_counter[0]}",
                            "opcode": "NoOp",
                            "engine": ins["engine"],
                            "ins": [],
                            "outs": [],
                            "sync_info": {"on_update": [], "on_wait": [w]},
                        })
                    si["on_wait"] = waits[-MAX_WAITS:]
                new_list.append(ins)
            b["instructions"] = new_list
        for sub in (b.get("blocks") or []):
            fix_block(sub)

    for fn in j.get("functions", []):
        for b in (fn.get("blocks") or []):
            fix_block(b)
    if not changed[0]:
        return raw
    return orjson.dumps(j)


_orig_to_json_bytes = bass.Bass.to_json_bytes


def _to_json_bytes_split(self) -> bytes:
    return _split_waits_json(_orig_to_json_bytes(self))


tile.TileContext._drain_and_barrier = _drain_and_barrier_split
bass.Bass.to_json_bytes = _to_json_bytes_split

# ---------------------------------------------------------------------------
# Kernel
# ---------------------------------------------------------------------------
P = 128
MAGIC = 12582912.0  # 1.5 * 2**23: fp32 add/sub rounds to nearest-even integer
FREE = 512          # matmul moving free dim / psum bank width

M_FULL, K_DIM, N_FULL = 8192, 4096, 12288
N_CORES = 8
NS = N_FULL // N_CORES  # 1536 out_features per core

f32 = mybir.dt.float32
fp16 = mybir.dt.float16
i8 = mybir.dt.int8


def _emit(ctx: ExitStack, tc: tile.TileContext, x_ap, w_ap, ws_ap, b_ap, out_ap):
    nc = tc.nc
    M, K = x_ap.shape
    NSl, K2 = w_ap.shape
    assert K == K2
    KT = K // P         # k chunks (32)
    MT = M // P         # m tiles (64)
    NB = NSl // FREE    # psum banks per m tile (3)
    NWB = NSl // P      # w row-blocks (12)

    const = ctx.enter_context(tc.tile_pool(name="const", bufs=1))

    # bias broadcast to all 128 partitions (one-time)
    bb = const.tile([P, NSl], f32)
    nc.scalar.dma_start(bb[:], b_ap[None, :].partition_broadcast(P))

    # per-channel scale as per-partition scalars: ws_sb[p, nb] = ws[nb*128+p]
    ws_sb = const.tile([P, NWB], f32)
    nc.scalar.dma_start(ws_sb[:], ws_ap.rearrange("(nb p) one -> p (nb one)", p=P))

    # -MAGIC as a per-partition bias vector for the ACT de-bias pass
    negmagic = const.tile([P, 1], f32)
    nc.vector.memset(negmagic[:], -MAGIC)

    # resident K-major scaled weight: wT[p, c, n] = w[n, c*128+p] * ws[n]
    wT = const.tile([P, KT, NSl], fp16)

    xpool = ctx.enter_context(tc.tile_pool(name="x", bufs=2))
    qpool = ctx.enter_context(tc.tile_pool(name="q", bufs=2))
    qtpool = ctx.enter_context(tc.tile_pool(name="qt", bufs=2))
    opool = ctx.enter_context(tc.tile_pool(name="o", bufs=2))
    spool = ctx.enter_context(tc.tile_pool(name="s", bufs=4))
    mpsum = ctx.enter_context(tc.tile_pool(name="mpsum", bufs=6, space="PSUM"))
    wraw = ctx.enter_context(tc.tile_pool(name="wraw", bufs=2))
    wstage = ctx.enter_context(tc.tile_pool(name="wst", bufs=2))

    def quant_chain(mi):
        """DMA + quantize + X-bar-transpose one x m-tile -> (xqT, xs)."""
        xt = xpool.tile([P, K], f32, tag="xt", name=f"xt{mi}")
        nc.scalar.dma_start(xt[:], x_ap[mi * P:(mi + 1) * P, :])

        amax = spool.tile([P, 1], f32, tag="amax", name=f"amax{mi}")
        nc.vector.tensor_reduce(
            amax[:], xt[:], axis=mybir.AxisListType.X,
            op=mybir.AluOpType.max, apply_absolute_value=True,
        )
        xs = spool.tile([P, 1], f32, tag="xs", name=f"xs{mi}")
        nc.vector.tensor_scalar(
            xs[:], amax[:], 1e-8, 1.0 / 127.0,
            op0=mybir.AluOpType.max, op1=mybir.AluOpType.mult,
        )
        inv = spool.tile([P, 1], f32, tag="inv", name=f"inv{mi}")
        nc.vector.reciprocal(inv[:], xs[:])

        # x_q = (x * inv + MAGIC) - MAGIC, cast to fp16 (exact for ints).
        # Pass 1 runs in place over the x tile (amax already consumed it).
        nc.vector.tensor_scalar(
            xt[:], xt[:], inv[:, 0:1], MAGIC,
            op0=mybir.AluOpType.mult, op1=mybir.AluOpType.add,
        )
        xq = qpool.tile([P, K], fp16, tag="xq", name=f"xq{mi}")
        nc.scalar.activation(
            xq[:], xt[:], mybir.ActivationFunctionType.Identity,
            bias=negmagic[:, 0:1],
        )

        # X-bar transpose: xqT[p, c, m] = xq[m, c*128+p]
        xqT = qtpool.tile([P, KT, P], fp16, tag="xqT", name=f"xqT{mi}")
        nc.sync.dma_start(xqT[:], xq[:], transpose=True)
        return xqT, xs

    # weight setup: raw int8 HWDGE load -> fused cast*ws on DVE -> X-bar
    # transpose. All transposes stay on the sync ring: mixing DMA-transpose
    # with plain copies on one HWDGE ring corrupts data (xbar-mode HW bug).
    def stage_w_block(nb_i):
        wr = wraw.tile([P, K], i8, tag="wraw", name=f"wraw{nb_i}")
        nc.scalar.dma_start(wr[:], w_ap[nb_i * P:(nb_i + 1) * P, :])
        wst = wstage.tile([P, K], fp16, tag="wst", name=f"wst{nb_i}")
        nc.vector.tensor_scalar(
            wst[:], wr[:], ws_sb[:, nb_i:nb_i + 1], None,
            op0=mybir.AluOpType.mult,
        )
        nc.sync.dma_start(wT[:, :, nb_i * P:(nb_i + 1) * P], wst[:],
                          transpose=True)

    # Emission order = scheduler priority: the first psum bank only needs
    # w blocks 0-3, so stage those, then warm the x pipeline, then stage
    # the rest of the weights while the first matmuls already run.
    for nb_i in range(4):
        stage_w_block(nb_i)
    NPRE = 2
    chains = {mi: quant_chain(mi) for mi in range(NPRE)}
    for nb_i in range(4, NWB):
        stage_w_block(nb_i)

    for mi in range(MT):
        if mi + NPRE < MT:
            chains[mi + NPRE] = quant_chain(mi + NPRE)
        xqT, xs = chains.pop(mi)

        # main matmuls, bank-outer: acc[m, n] += xq[m, c*128+p] * wT[p, c, n];
        # each bank's epilogue overlaps the next bank's accumulation
        ot = opool.tile([P, NSl], f32, tag="ot", name=f"ot{mi}")
        for nb_i in range(NB):
            bank = mpsum.tile([P, FREE], f32, tag="mps", name=f"mps{mi}_{nb_i}")
            for c in range(KT):
                nc.tensor.matmul(
                    bank[:],
                    xqT[:, c, :],
                    wT[:, c, nb_i * FREE:(nb_i + 1) * FREE],
                    start=(c == 0), stop=(c == KT - 1),
                )
            # epilogue: out = acc * xs + b   (ws already folded into wT)
            sl = slice(nb_i * FREE, (nb_i + 1) * FREE)
            nc.vector.scalar_tensor_tensor(
                ot[:, sl], bank[:], xs[:, 0:1], bb[:, sl],
                op0=mybir.AluOpType.mult, op1=mybir.AluOpType.add,
            )
        nc.scalar.dma_start(out_ap[mi * P:(mi + 1) * P, :], ot[:])


def _build_nc():
    nc = bass.Bass()
    x = nc.dram_tensor("x", (M_FULL, K_DIM), f32, kind="ExternalInput")
    w = nc.dram_tensor("w", (NS, K_DIM), i8, kind="ExternalInput")
    ws = nc.dram_tensor("ws", (NS, 1), f32, kind="ExternalInput")
    b = nc.dram_tensor("b", (NS,), f32, kind="ExternalInput")
    out = nc.dram_tensor("out", (M_FULL, NS), f32, kind="ExternalOutput")
    with tile.TileContext(nc) as tc:
        with ExitStack() as ctx:
            _emit(ctx, tc, x[:], w[:], ws[:], b[:], out[:])
    return nc


_nc_cache = None


def _get_nc():
    global _nc_cache
    if _nc_cache is None:
        _nc_cache = _build_nc()
    return _nc_cache


def _in_maps(x, weight, weight_scale, bias):
    in_maps = []
    for c in range(N_CORES):
        sl = slice(c * NS, (c + 1) * NS)
        in_maps.append({
            "x": x,
            "w": weight[sl],
            "ws": weight_scale[sl],
            "b": bias[sl],
        })
    return in_maps


def _run(nc, in_maps, **kwargs):
    return run_bass_kernel_spmd(nc, in_maps, core_ids=list(range(N_CORES)), **kwargs)


def kernel(x, weight, weight_scale, bias):
    x = np.ascontiguousarray(np.asarray(x, dtype=np.float32))
    weight = np.ascontiguousarray(np.asarray(weight, dtype=np.int8))
    weight_scale = np.ascontiguousarray(np.asarray(weight_scale, dtype=np.float32))
    bias = np.ascontiguousarray(np.asarray(bias, dtype=np.float32))
    assert x.shape == (M_FULL, K_DIM)
    assert weight.shape == (N_FULL, K_DIM)

    nc = _get_nc()
    res = _run(nc, _in_maps(x, weight, weight_scale, bias))
    out = np.concatenate([res.results[c]["out"] for c in range(N_CORES)], axis=1)
    return out.astype(np.float32)


# revision 20
# speedup vs baseline: 1.0109x; 1.0109x over previous
"""DynamicW8A8Int8Linear on 8 Trainium2 NeuronCores (Bass/Tile).

Column-parallel (tensor-parallel on out_features): each core gets the full
activation x [8192, 4096] and a 1536-wide shard of weight / weight_scale /
bias; it computes its [8192, 1536] slice of the output. No communication.

Per-core pipeline (per 128-token m-tile):
  - DMA x tile [128, 4096] fp32 (HWDGE, ACT ring)
  - GpSimd: amax = max|x| over K;  DVE: xs = max(amax,1e-8)/127, inv = 1/xs
  - DVE: t = x*inv + MAGIC (fp32 magic-number round-half-even)
  - ACT: xq = t - MAGIC, cast to fp16 (ints in [-127,127] are exact)
  - DMA X-bar transpose (SP ring): xqT[p, c, m] = xq[m, c*128+p] -- the
    K-major layout the PE needs, at ~290 GB/s, zero PE/DVE cost
  - PE: 3 psum banks x 32 accumulating fp16 matmuls (512-wide moving op)
  - DVE: out = xs*acc + bias (one scalar_tensor_tensor per bank)
The weight shard is staged once: int8 -> fp16 cast during DMA (SWDGE),
multiplied by its per-channel scale (so the epilogue needs no ws), then
X-bar-transposed into the resident K-major wT tile reused by every m-tile.
fp16 keeps w*ws rounding at 2^-11 and the fp32-PSUM accumulation exact
enough for ~1e-3 relative error.
"""

import sys
from contextlib import ExitStack

import numpy as np

for p in ("/opt/trn_rl_repo", "/opt/pypackages"):
    if p not in sys.path:
        sys.path.append(p)

import orjson
import bass_rust
import concourse.bass as bass
import concourse.mybir as mybir
import concourse.tile as tile
from concourse.vector_clock import ScopedClock
from concourse.bass_utils import run_bass_kernel_spmd

# ---------------------------------------------------------------------------
# Workaround for the walrus build here, which accepts at most ONE sem-wait per
# instruction ("Too many sync wait commands" in setupSyncWait): split the Tile
# end-drain at emission time, and hoist excess waits from any instruction onto
# injected same-engine NoOps at serialization time (program order on the same
# engine makes that semantically identical).
# ---------------------------------------------------------------------------
MAX_WAITS = 1


def _drain_and_barrier_split(self, tick_clock, wait_clock):
    nc = self.nc
    drain_inst = nc.sync.drain()
    wait_clock.add_sem_waits(drain_inst.ins, ScopedClock({None: tick_clock.global_clock}))
    si = drain_inst.ins.sync_info
    waits = list(si.on_wait) if si is not None and si.on_wait else []
    if len(waits) > MAX_WAITS:
        si.on_wait = waits[:MAX_WAITS]
        drain_inst.ins.sync_info = si
        rest = waits[MAX_WAITS:]
        while rest:
            extra = nc.sync.drain()
            extra.ins.sync_info = bass_rust.SyncInfo(
                on_wait=rest[:MAX_WAITS], on_update=[])
            rest = rest[MAX_WAITS:]
    nc.all_engine_barrier()
    assert self.sems is not None
    popped = nc._tile_sem_poison_stack.pop()
    assert popped is self._sem_poison
    nc.clear_and_free_semaphores(list(self.sems.allocated().values()))
    nc.all_engine_barrier()


_split_counter = [0]


def _split_waits_json(raw: bytes) -> bytes:
    j = orjson.loads(raw)
    changed = [False]

    def fix_block(b):
        ins_list = b.get("instructions")
        if ins_list:
            new_list = []
            for ins in ins_list:
                si = ins.get("sync_info")
                waits = (si or {}).get("on_wait") or []
                if len(waits) > MAX_WAITS:
                    changed[0] = True
                    for w in waits[:-MAX_WAITS]:
                        _split_counter[0] += 1
                        new_list.append({
                            "name": f"WSPLIT-{_split_counter[0]}",
                            "opcode": "NoOp",
                            "engine": ins["engine"],
                            "ins": [],
                            "outs": [],
                            "sync_info": {"on_update": [], "on_wait": [w]},
                        })
                    si["on_wait"] = waits[-MAX_WAITS:]
                new_list.append(ins)
            b["instructions"] = new_list
        for sub in (b.get("blocks") or []):
            fix_block(sub)

    for fn in j.get("functions", []):
        for b in (fn.get("blocks") or []):
            fix_block(b)
    if not changed[0]:
        return raw
    return orjson.dumps(j)


_orig_to_json_bytes = bass.Bass.to_json_bytes


def _to_json_bytes_split(self) -> bytes:
    return _split_waits_json(_orig_to_json_bytes(self))


tile.TileContext._drain_and_barrier = _drain_and_barrier_split
bass.Bass.to_json_bytes = _to_json_bytes_split

# ---------------------------------------------------------------------------
# Kernel
# ---------------------------------------------------------------------------
P = 128
MAGIC = 12582912.0  # 1.5 * 2**23: fp32 add/sub rounds to nearest-even integer
FREE = 512          # matmul moving free dim / psum bank width

M_FULL, K_DIM, N_FULL = 8192, 4096, 12288
N_CORES = 8
NS = N_FULL // N_CORES  # 1536 out_features per core

f32 = mybir.dt.float32
fp16 = mybir.dt.float16
i8 = mybir.dt.int8


def _emit(ctx: ExitStack, tc: tile.TileContext, x_ap, w_ap, ws_ap, b_ap, out_ap):
    nc = tc.nc
    M, K = x_ap.shape
    NSl, K2 = w_ap.shape
    assert K == K2
    KT = K // P         # k chunks (32)
    MT = M // P         # m tiles (64)
    NB = NSl // FREE    # psum banks per m tile (3)
    NWB = NSl // P      # w row-blocks (12)

    const = ctx.enter_context(tc.tile_pool(name="const", bufs=1))

    # bias broadcast to all 128 partitions (one-time; fp16 to save SBUF,
    # costs at most 2.5e-5 absolute error; SWDGE because HWDGE can't cast)
    bb = const.tile([P, NSl], fp16)
    nc.gpsimd.dma_start(bb[:], b_ap[None, :].partition_broadcast(P))

    # per-channel scale as per-partition scalars: ws_sb[p, nb] = ws[nb*128+p]
    ws_sb = const.tile([P, NWB], f32)
    nc.scalar.dma_start(ws_sb[:], ws_ap.rearrange("(nb p) one -> p (nb one)", p=P))

    # -MAGIC as a per-partition bias vector for the ACT de-bias pass
    negmagic = const.tile([P, 1], f32)
    nc.vector.memset(negmagic[:], -MAGIC)

    # resident K-major scaled weight: wT[p, c, n] = w[n, c*128+p] * ws[n]
    wT = const.tile([P, KT, NSl], fp16)

    xpool = ctx.enter_context(tc.tile_pool(name="x", bufs=2))
    qpool = ctx.enter_context(tc.tile_pool(name="q", bufs=2))
    qtpool = ctx.enter_context(tc.tile_pool(name="qt", bufs=2))
    opool = ctx.enter_context(tc.tile_pool(name="o", bufs=2))
    spool = ctx.enter_context(tc.tile_pool(name="s", bufs=4))
    mpsum = ctx.enter_context(tc.tile_pool(name="mpsum", bufs=6, space="PSUM"))
    wraw = ctx.enter_context(tc.tile_pool(name="wraw", bufs=1))
    wstage = ctx.enter_context(tc.tile_pool(name="wst", bufs=2))

    # preload the ACT function table so the first real ACTIVATE doesn't
    # pay the ~2.7us table load on the critical path
    actwarm = const.tile([P, 1], f32)
    nc.scalar.activation(
        actwarm[:], negmagic[:], mybir.ActivationFunctionType.Identity,
        bias=negmagic[:, 0:1],
    )

    def quant_chain(mi):
        """DMA + quantize + X-bar-transpose one x m-tile -> (xqT, xs)."""
        xt = xpool.tile([P, K], f32, tag="xt", name=f"xt{mi}")
        nc.scalar.dma_start(xt[:], x_ap[mi * P:(mi + 1) * P, :])

        amax = spool.tile([P, 1], f32, tag="amax", name=f"amax{mi}")
        nc.vector.tensor_reduce(
            amax[:], xt[:], axis=mybir.AxisListType.X,
            op=mybir.AluOpType.max, apply_absolute_value=True,
        )
        xs = spool.tile([P, 1], f32, tag="xs", name=f"xs{mi}")
        nc.vector.tensor_scalar(
            xs[:], amax[:], 1e-8, 1.0 / 127.0,
            op0=mybir.AluOpType.max, op1=mybir.AluOpType.mult,
        )
        inv = spool.tile([P, 1], f32, tag="inv", name=f"inv{mi}")
        nc.vector.reciprocal(inv[:], xs[:])

        # x_q = (x * inv + MAGIC) - MAGIC, cast to fp16 (exact for ints).
        # Pass 1 runs in place over the x tile (amax already consumed it).
        nc.vector.tensor_scalar(
            xt[:], xt[:], inv[:, 0:1], MAGIC,
            op0=mybir.AluOpType.mult, op1=mybir.AluOpType.add,
        )
        xq = qpool.tile([P, K], fp16, tag="xq", name=f"xq{mi}")
        nc.scalar.activation(
            xq[:], xt[:], mybir.ActivationFunctionType.Identity,
            bias=negmagic[:, 0:1],
        )

        # X-bar transpose: xqT[p, c, m] = xq[m, c*128+p]
        xqT = qtpool.tile([P, KT, P], fp16, tag="xqT", name=f"xqT{mi}")
        nc.sync.dma_start(xqT[:], xq[:], transpose=True)
        return xqT, xs

    # weight setup: one strided HWDGE load per 4-block group -> fused
    # cast*ws on DVE per block -> X-bar transpose. All transposes stay on
    # the sync ring: mixing DMA-transpose with plain copies on one HWDGE
    # ring corrupts data (xbar-mode HW bug).
    def stage_w_group(g):
        wr = wraw.tile([P, 4, K], i8, tag="wraw", name=f"wraw{g}")
        nc.scalar.dma_start(
            wr[:],
            w_ap[g * 4 * P:(g + 1) * 4 * P, :].rearrange(
                "(nb p) k -> p nb k", p=P),
        )
        for j in range(4):
            nb_i = g * 4 + j
            wst = wstage.tile([P, K], fp16, tag="wst", name=f"wst{nb_i}")
            nc.vector.tensor_scalar(
                wst[:], wr[:, j, :], ws_sb[:, nb_i:nb_i + 1], None,
                op0=mybir.AluOpType.mult,
            )
            nc.sync.dma_start(wT[:, :, nb_i * P:(nb_i + 1) * P], wst[:],
                              transpose=True)

    def mm_bank(bank, xqT, nb_i):
        for c in range(KT):
            nc.tensor.matmul(
                bank[:],
                xqT[:, c, :],
                wT[:, c, nb_i * FREE:(nb_i + 1) * FREE],
                start=(c == 0), stop=(c == KT - 1),
            )

    def epi_bank(ot, bank, xs, nb_i):
        # epilogue: out = acc * xs + b   (ws already folded into wT)
        sl = slice(nb_i * FREE, (nb_i + 1) * FREE)
        nc.vector.scalar_tensor_tensor(
            ot[:, sl], bank[:], xs[:, 0:1], bb[:, sl],
            op0=mybir.AluOpType.mult, op1=mybir.AluOpType.add,
        )

    # Emission order = scheduler priority: the first psum bank only needs
    # w blocks 0-3, so stage those, then warm the x pipeline, then stage
    # the rest of the weights while the first matmuls already run.
    stage_w_group(0)
    chains = {mi: quant_chain(mi) for mi in range(2)}
    stage_w_group(1)
    stage_w_group(2)

    # First two tiles run bank-major so the matmuls chase the weight
    # pipeline group by group instead of stalling on the last blocks.
    ots = {mi: opool.tile([P, NSl], f32, tag="ot", name=f"ot{mi}")
           for mi in (0, 1)}
    extra = iter([m for m in (2, 3) if m < MT])
    for nb_i in range(NB):
        for mi in (0, 1):
            bank = mpsum.tile([P, FREE], f32, tag="mps", name=f"mps{mi}_{nb_i}")
            mm_bank(bank, chains[mi][0], nb_i)
            epi_bank(ots[mi], bank, chains[mi][1], nb_i)
        nxt = next(extra, None)
        if nxt is not None:
            chains[nxt] = quant_chain(nxt)
    for mi in (0, 1):
        nc.scalar.dma_start(out_ap[mi * P:(mi + 1) * P, :], ots[mi][:])
        chains.pop(mi)

    NPRE = 2
    for mi in range(2, MT):
        if mi + NPRE < MT:
            chains[mi + NPRE] = quant_chain(mi + NPRE)
        xqT, xs = chains.pop(mi)

        # main matmuls, bank-outer: acc[m, n] += xq[m, c*128+p] * wT[p, c, n];
        # each bank's epilogue overlaps the next bank's accumulation
        ot = opool.tile([P, NSl], f32, tag="ot", name=f"ot{mi}")
        for nb_i in range(NB):
            bank = mpsum.tile([P, FREE], f32, tag="mps", name=f"mps{mi}_{nb_i}")
            mm_bank(bank, xqT, nb_i)
            epi_bank(ot, bank, xs, nb_i)
        nc.scalar.dma_start(out_ap[mi * P:(mi + 1) * P, :], ot[:])


def _build_nc():
    nc = bass.Bass()
    x = nc.dram_tensor("x", (M_FULL, K_DIM), f32, kind="ExternalInput")
    w = nc.dram_tensor("w", (NS, K_DIM), i8, kind="ExternalInput")
    ws = nc.dram_tensor("ws", (NS, 1), f32, kind="ExternalInput")
    b = nc.dram_tensor("b", (NS,), f32, kind="ExternalInput")
    out = nc.dram_tensor("out", (M_FULL, NS), f32, kind="ExternalOutput")
    with tile.TileContext(nc) as tc:
        with ExitStack() as ctx:
            _emit(ctx, tc, x[:], w[:], ws[:], b[:], out[:])
    return nc


_nc_cache = None


def _get_nc():
    global _nc_cache
    if _nc_cache is None:
        _nc_cache = _build_nc()
    return _nc_cache


def _in_maps(x, weight, weight_scale, bias):
    in_maps = []
    for c in range(N_CORES):
        sl = slice(c * NS, (c + 1) * NS)
        in_maps.append({
            "x": x,
            "w": weight[sl],
            "ws": weight_scale[sl],
            "b": bias[sl],
        })
    return in_maps


def _run(nc, in_maps, **kwargs):
    return run_bass_kernel_spmd(nc, in_maps, core_ids=list(range(N_CORES)), **kwargs)


def kernel(x, weight, weight_scale, bias):
    x = np.ascontiguousarray(np.asarray(x, dtype=np.float32))
    weight = np.ascontiguousarray(np.asarray(weight, dtype=np.int8))
    weight_scale = np.ascontiguousarray(np.asarray(weight_scale, dtype=np.float32))
    bias = np.ascontiguousarray(np.asarray(bias, dtype=np.float32))
    assert x.shape == (M_FULL, K_DIM)
    assert weight.shape == (N_FULL, K_DIM)

    nc = _get_nc()
    res = _run(nc, _in_maps(x, weight, weight_scale, bias))
    out = np.concatenate([res.results[c]["out"] for c in range(N_CORES)], axis=1)
    return out.astype(np.float32)


# revision 25
# speedup vs baseline: 1.0255x; 1.0145x over previous
"""DynamicW8A8Int8Linear on 8 Trainium2 NeuronCores (Bass/Tile).

Column-parallel (tensor-parallel on out_features): each core gets the full
activation x [8192, 4096] and a 1536-wide shard of weight / weight_scale /
bias; it computes its [8192, 1536] slice of the output. No communication.

Per-core pipeline (per 128-token m-tile):
  - DMA x tile [128, 4096] fp32 (HWDGE, ACT ring)
  - GpSimd: amax = max|x| over K;  DVE: xs = max(amax,1e-8)/127, inv = 1/xs
  - DVE: t = x*inv + MAGIC (fp32 magic-number round-half-even)
  - ACT: xq = t - MAGIC, cast to fp16 (ints in [-127,127] are exact)
  - DMA X-bar transpose (SP ring): xqT[p, c, m] = xq[m, c*128+p] -- the
    K-major layout the PE needs, at ~290 GB/s, zero PE/DVE cost
  - PE: 3 psum banks x 32 accumulating fp16 matmuls (512-wide moving op)
  - DVE: out = xs*acc + bias (one scalar_tensor_tensor per bank)
The weight shard is staged once: int8 -> fp16 cast during DMA (SWDGE),
multiplied by its per-channel scale (so the epilogue needs no ws), then
X-bar-transposed into the resident K-major wT tile reused by every m-tile.
fp16 keeps w*ws rounding at 2^-11 and the fp32-PSUM accumulation exact
enough for ~1e-3 relative error.
"""

import sys
from contextlib import ExitStack

import numpy as np

for p in ("/opt/trn_rl_repo", "/opt/pypackages"):
    if p not in sys.path:
        sys.path.append(p)

import orjson
import bass_rust
import concourse.bass as bass
import concourse.mybir as mybir
import concourse.tile as tile
from concourse.vector_clock import ScopedClock
from concourse.bass_utils import run_bass_kernel_spmd

# ---------------------------------------------------------------------------
# Workaround for the walrus build here, which accepts at most ONE sem-wait per
# instruction ("Too many sync wait commands" in setupSyncWait): split the Tile
# end-drain at emission time, and hoist excess waits from any instruction onto
# injected same-engine NoOps at serialization time (program order on the same
# engine makes that semantically identical).
# ---------------------------------------------------------------------------
MAX_WAITS = 1


def _drain_and_barrier_split(self, tick_clock, wait_clock):
    nc = self.nc
    drain_inst = nc.sync.drain()
    wait_clock.add_sem_waits(drain_inst.ins, ScopedClock({None: tick_clock.global_clock}))
    si = drain_inst.ins.sync_info
    waits = list(si.on_wait) if si is not None and si.on_wait else []
    if len(waits) > MAX_WAITS:
        si.on_wait = waits[:MAX_WAITS]
        drain_inst.ins.sync_info = si
        rest = waits[MAX_WAITS:]
        while rest:
            extra = nc.sync.drain()
            extra.ins.sync_info = bass_rust.SyncInfo(
                on_wait=rest[:MAX_WAITS], on_update=[])
            rest = rest[MAX_WAITS:]
    nc.all_engine_barrier()
    assert self.sems is not None
    popped = nc._tile_sem_poison_stack.pop()
    assert popped is self._sem_poison
    nc.clear_and_free_semaphores(list(self.sems.allocated().values()))
    nc.all_engine_barrier()


_split_counter = [0]


def _split_waits_json(raw: bytes) -> bytes:
    j = orjson.loads(raw)
    changed = [False]

    def fix_block(b):
        ins_list = b.get("instructions")
        if ins_list:
            new_list = []
            for ins in ins_list:
                si = ins.get("sync_info")
                waits = (si or {}).get("on_wait") or []
                if len(waits) > MAX_WAITS:
                    changed[0] = True
                    for w in waits[:-MAX_WAITS]:
                        _split_counter[0] += 1
                        new_list.append({
                            "name": f"WSPLIT-{_split_counter[0]}",
                            "opcode": "NoOp",
                            "engine": ins["engine"],
                            "ins": [],
                            "outs": [],
                            "sync_info": {"on_update": [], "on_wait": [w]},
                        })
                    si["on_wait"] = waits[-MAX_WAITS:]
                new_list.append(ins)
            b["instructions"] = new_list
        for sub in (b.get("blocks") or []):
            fix_block(sub)

    for fn in j.get("functions", []):
        for b in (fn.get("blocks") or []):
            fix_block(b)
    if not changed[0]:
        return raw
    return orjson.dumps(j)


_orig_to_json_bytes = bass.Bass.to_json_bytes


def _to_json_bytes_split(self) -> bytes:
    return _split_waits_json(_orig_to_json_bytes(self))


tile.TileContext._drain_and_barrier = _drain_and_barrier_split
bass.Bass.to_json_bytes = _to_json_bytes_split

# ---------------------------------------------------------------------------
# Kernel
# ---------------------------------------------------------------------------
P = 128
MAGIC = 12582912.0  # 1.5 * 2**23: fp32 add/sub rounds to nearest-even integer
FREE = 512          # matmul moving free dim / psum bank width

M_FULL, K_DIM, N_FULL = 8192, 4096, 12288
N_CORES = 8
NS = N_FULL // N_CORES  # 1536 out_features per core

f32 = mybir.dt.float32
fp16 = mybir.dt.float16
i8 = mybir.dt.int8


def _emit(ctx: ExitStack, tc: tile.TileContext, x_ap, w_ap, ws_ap, b_ap, out_ap):
    nc = tc.nc
    M, K = x_ap.shape
    NSl, K2 = w_ap.shape
    assert K == K2
    KT = K // P         # k chunks (32)
    MT = M // P         # m tiles (64)
    NB = NSl // FREE    # psum banks per m tile (3)
    NWB = NSl // P      # w row-blocks (12)

    const = ctx.enter_context(tc.tile_pool(name="const", bufs=1))

    # bias broadcast to all 128 partitions (one-time; fp16 to save SBUF,
    # costs at most 2.5e-5 absolute error; SWDGE because HWDGE can't cast)
    bb = const.tile([P, NSl], fp16)
    nc.gpsimd.dma_start(bb[:], b_ap[None, :].partition_broadcast(P))

    # per-channel scale as per-partition scalars: ws_sb[p, nb] = ws[nb*128+p]
    ws_sb = const.tile([P, NWB], f32)
    nc.scalar.dma_start(ws_sb[:], ws_ap.rearrange("(nb p) one -> p (nb one)", p=P))

    # -MAGIC as a per-partition bias vector for the ACT de-bias pass
    negmagic = const.tile([P, 1], f32)
    nc.vector.memset(negmagic[:], -MAGIC)

    # resident K-major scaled weight: wT[p, c, n] = w[n, c*128+p] * ws[n]
    wT = const.tile([P, KT, NSl], fp16)

    xpool = ctx.enter_context(tc.tile_pool(name="x", bufs=2))
    qpool = ctx.enter_context(tc.tile_pool(name="q", bufs=2))
    qtpool = ctx.enter_context(tc.tile_pool(name="qt", bufs=2))
    opool = ctx.enter_context(tc.tile_pool(name="o", bufs=2))
    spool = ctx.enter_context(tc.tile_pool(name="s", bufs=4))
    mpsum = ctx.enter_context(tc.tile_pool(name="mpsum", bufs=6, space="PSUM"))
    wraw = ctx.enter_context(tc.tile_pool(name="wraw", bufs=1))
    wstage = ctx.enter_context(tc.tile_pool(name="wst", bufs=3))

    # preload the ACT function table so the first real ACTIVATE doesn't
    # pay the ~2.7us table load on the critical path
    actwarm = const.tile([P, 1], f32)
    nc.scalar.activation(
        actwarm[:], negmagic[:], mybir.ActivationFunctionType.Identity,
        bias=negmagic[:, 0:1],
    )

    def quant_chain(mi, ld_eng=None):
        """DMA + quantize + X-bar-transpose one x m-tile -> (xqT, xs)."""
        xt = xpool.tile([P, K], f32, tag="xt", name=f"xt{mi}")
        (ld_eng or nc.scalar).dma_start(xt[:], x_ap[mi * P:(mi + 1) * P, :])

        amax = spool.tile([P, 1], f32, tag="amax", name=f"amax{mi}")
        nc.vector.tensor_reduce(
            amax[:], xt[:], axis=mybir.AxisListType.X,
            op=mybir.AluOpType.max, apply_absolute_value=True,
        )
        xs = spool.tile([P, 1], f32, tag="xs", name=f"xs{mi}")
        nc.vector.tensor_scalar(
            xs[:], amax[:], 1e-8, 1.0 / 127.0,
            op0=mybir.AluOpType.max, op1=mybir.AluOpType.mult,
        )
        inv = spool.tile([P, 1], f32, tag="inv", name=f"inv{mi}")
        nc.vector.reciprocal(inv[:], xs[:])

        # x_q = (x * inv + MAGIC) - MAGIC, cast to fp16 (exact for ints).
        # Pass 1 runs in place over the x tile (amax already consumed it).
        nc.vector.tensor_scalar(
            xt[:], xt[:], inv[:, 0:1], MAGIC,
            op0=mybir.AluOpType.mult, op1=mybir.AluOpType.add,
        )
        xq = qpool.tile([P, K], fp16, tag="xq", name=f"xq{mi}")
        nc.scalar.activation(
            xq[:], xt[:], mybir.ActivationFunctionType.Identity,
            bias=negmagic[:, 0:1],
        )

        # X-bar transpose: xqT[p, c, m] = xq[m, c*128+p]
        xqT = qtpool.tile([P, KT, P], fp16, tag="xqT", name=f"xqT{mi}")
        nc.sync.dma_start(xqT[:], xq[:], transpose=True)
        return xqT, xs

    # weight setup: one strided HWDGE load per 2-block pair -> fused
    # cast*ws on DVE per block -> X-bar transpose. All transposes stay on
    # the sync ring: mixing DMA-transpose with plain copies on one HWDGE
    # ring corrupts data (xbar-mode HW bug).
    def stage_w_group(g):
        wr = wraw.tile([P, 2, K], i8, tag="wraw", name=f"wraw{g}")
        nc.scalar.dma_start(
            wr[:],
            w_ap[g * 2 * P:(g + 1) * 2 * P, :].rearrange(
                "(nb p) k -> p nb k", p=P),
        )
        for j in range(2):
            nb_i = g * 2 + j
            wst = wstage.tile([P, K], fp16, tag="wst", name=f"wst{nb_i}")
            nc.vector.tensor_scalar(
                wst[:], wr[:, j, :], ws_sb[:, nb_i:nb_i + 1], None,
                op0=mybir.AluOpType.mult,
            )
            nc.sync.dma_start(wT[:, :, nb_i * P:(nb_i + 1) * P], wst[:],
                              transpose=True)

    def mm_bank(bank, xqT, nb_i):
        for c in range(KT):
            nc.tensor.matmul(
                bank[:],
                xqT[:, c, :],
                wT[:, c, nb_i * FREE:(nb_i + 1) * FREE],
                start=(c == 0), stop=(c == KT - 1),
            )

    def epi_bank(ot, bank, xs, nb_i):
        # epilogue: out = acc * xs + b   (ws already folded into wT)
        sl = slice(nb_i * FREE, (nb_i + 1) * FREE)
        nc.vector.scalar_tensor_tensor(
            ot[:, sl], bank[:], xs[:, 0:1], bb[:, sl],
            op0=mybir.AluOpType.mult, op1=mybir.AluOpType.add,
        )

    # Emission order = scheduler priority: the first psum bank only needs
    # w blocks 0-3, so stage those, then warm the x pipeline, then stage
    # the rest of the weights while the first matmuls already run.
    stage_w_group(0)
    stage_w_group(1)
    chains = {mi: quant_chain(mi, nc.gpsimd) for mi in range(2)}
    for g in range(2, NWB // 2):
        stage_w_group(g)

    # First two tiles run bank-major so the matmuls chase the weight
    # pipeline group by group instead of stalling on the last blocks.
    ots = {mi: opool.tile([P, NSl], f32, tag="ot", name=f"ot{mi}")
           for mi in (0, 1)}
    extra = iter([m for m in (2, 3) if m < MT])
    for nb_i in range(NB):
        for mi in (0, 1):
            bank = mpsum.tile([P, FREE], f32, tag="mps", name=f"mps{mi}_{nb_i}")
            mm_bank(bank, chains[mi][0], nb_i)
            epi_bank(ots[mi], bank, chains[mi][1], nb_i)
        nxt = next(extra, None)
        if nxt is not None:
            chains[nxt] = quant_chain(nxt)
    for mi in (0, 1):
        nc.scalar.dma_start(out_ap[mi * P:(mi + 1) * P, :], ots[mi][:])
        chains.pop(mi)

    NPRE = 2
    for mi in range(2, MT):
        if mi + NPRE < MT:
            chains[mi + NPRE] = quant_chain(mi + NPRE)
        xqT, xs = chains.pop(mi)

        # main matmuls, bank-outer: acc[m, n] += xq[m, c*128+p] * wT[p, c, n];
        # each bank's epilogue overlaps the next bank's accumulation
        ot = opool.tile([P, NSl], f32, tag="ot", name=f"ot{mi}")
        for nb_i in range(NB):
            bank = mpsum.tile([P, FREE], f32, tag="mps", name=f"mps{mi}_{nb_i}")
            mm_bank(bank, xqT, nb_i)
            epi_bank(ot, bank, xs, nb_i)
        nc.scalar.dma_start(out_ap[mi * P:(mi + 1) * P, :], ot[:])


def _build_nc():
    nc = bass.Bass()
    x = nc.dram_tensor("x", (M_FULL, K_DIM), f32, kind="ExternalInput")
    w = nc.dram_tensor("w", (NS, K_DIM), i8, kind="ExternalInput")
    ws = nc.dram_tensor("ws", (NS, 1), f32, kind="ExternalInput")
    b = nc.dram_tensor("b", (NS,), f32, kind="ExternalInput")
    out = nc.dram_tensor("out", (M_FULL, NS), f32, kind="ExternalOutput")
    with tile.TileContext(nc) as tc:
        with ExitStack() as ctx:
            _emit(ctx, tc, x[:], w[:], ws[:], b[:], out[:])
    return nc


_nc_cache = None


def _get_nc():
    global _nc_cache
    if _nc_cache is None:
        _nc_cache = _build_nc()
    return _nc_cache


def _in_maps(x, weight, weight_scale, bias):
    in_maps = []
    for c in range(N_CORES):
        sl = slice(c * NS, (c + 1) * NS)
        in_maps.append({
            "x": x,
            "w": weight[sl],
            "ws": weight_scale[sl],
            "b": bias[sl],
        })
    return in_maps


def _run(nc, in_maps, **kwargs):
    return run_bass_kernel_spmd(nc, in_maps, core_ids=list(range(N_CORES)), **kwargs)


def kernel(x, weight, weight_scale, bias):
    x = np.ascontiguousarray(np.asarray(x, dtype=np.float32))
    weight = np.ascontiguousarray(np.asarray(weight, dtype=np.int8))
    weight_scale = np.ascontiguousarray(np.asarray(weight_scale, dtype=np.float32))
    bias = np.ascontiguousarray(np.asarray(bias, dtype=np.float32))
    assert x.shape == (M_FULL, K_DIM)
    assert weight.shape == (N_FULL, K_DIM)

    nc = _get_nc()
    res = _run(nc, _in_maps(x, weight, weight_scale, bias))
    out = np.concatenate([res.results[c]["out"] for c in range(N_CORES)], axis=1)
    return out.astype(np.float32)
